# revision 1
# baseline (speedup 1.0000x reference)
"""Causal multi-head self-attention on 8 Trainium2 NeuronCores.

Sharding: 4 batches x 2 head-groups (8 heads each). Core c = (b, g) with
b = c // 2, g = c % 2. Each core computes QKV projections for its weight
row-slice, attention for its 8 heads, and a partial out-projection
(Megatron row-parallel). Host sums the two partials per batch and adds bo.

All shapes hardcoded for x [4, 2048, 1024], 16 heads, head_dim 64, fp32.
"""

import sys
import numpy as np

if "/opt/trn_rl_repo" not in sys.path:
    sys.path.insert(0, "/opt/trn_rl_repo")

B = 4
S = 2048
D = 1024
HG = 2            # head groups (cores per batch)
NHL = 8           # heads per core
DH = 64
DG = NHL * DH     # 512 feature dims per core
SB = 512          # s-block
NSB = S // SB     # 4
NEG = -1.0e9
SCALE = 0.125     # 1/sqrt(64)

_CACHE = {}


def _build_nc():
    import concourse.bass as bass
    import concourse.bacc as bacc
    import concourse.tile as tile
    from concourse import mybir
    from contextlib import ExitStack

    f32 = mybir.dt.float32
    bf16 = mybir.dt.bfloat16
    AF = mybir.ActivationFunctionType
    ts = bass.ts

    nc = bacc.Bacc(None, target_bir_lowering=False)

    xt_d = nc.dram_tensor("xt", [D, S], bf16, kind="ExternalInput")
    wqt_d = nc.dram_tensor("wqt", [D, DG], bf16, kind="ExternalInput")
    wkt_d = nc.dram_tensor("wkt", [D, DG], bf16, kind="ExternalInput")
    wvt_d = nc.dram_tensor("wvt", [D, DG], bf16, kind="ExternalInput")
    wot_d = nc.dram_tensor("wot", [DG, D], bf16, kind="ExternalInput")
    bqr_d = nc.dram_tensor("bqr", [1, DG], bf16, kind="ExternalInput")
    bkr_d = nc.dram_tensor("bkr", [1, DG], bf16, kind="ExternalInput")
    bvt_d = nc.dram_tensor("bvt", [1, DG], bf16, kind="ExternalInput")
    out_d = nc.dram_tensor("out", [S, D], f32, kind="ExternalOutput")

    with tile.TileContext(nc) as tc, ExitStack() as ctx:
        consts = ctx.enter_context(tc.tile_pool(name="consts", bufs=1))
        cache = ctx.enter_context(tc.tile_pool(name="cache", bufs=1))
        xt_pool = ctx.enter_context(tc.tile_pool(name="xtp", bufs=1))
        qt_pool = ctx.enter_context(tc.tile_pool(name="qtp", bufs=1))
        work = ctx.enter_context(tc.tile_pool(name="work", bufs=1))
        ppool = ctx.enter_context(tc.tile_pool(name="pp", bufs=2, space="PSUM"))
        pscore = ctx.enter_context(tc.tile_pool(name="ps", bufs=4, space="PSUM"))
        pout2 = ctx.enter_context(tc.tile_pool(name="po", bufs=2, space="PSUM"))

        # ---- constants / weights in SBUF ----
        wq_t = consts.tile([128, 8, DG], bf16)
        wk_t = consts.tile([128, 8, DG], bf16)
        wv_t = consts.tile([128, 8, DG], bf16)
        wo_t = consts.tile([128, 4, D], bf16)
        for ec in range(8):
            nc.sync.dma_start(wq_t[:, ec, :], wqt_d[ts(ec, 128), :])
            nc.sync.dma_start(wk_t[:, ec, :], wkt_d[ts(ec, 128), :])
            nc.sync.dma_start(wv_t[:, ec, :], wvt_d[ts(ec, 128), :])
        for p in range(4):
            nc.sync.dma_start(wo_t[:, p, :], wot_d[ts(p, 128), :])
        bqr_t = consts.tile([1, DG], bf16)
        bkr_t = consts.tile([1, DG], bf16)
        bvt_t = consts.tile([1, DG], bf16)
        nc.sync.dma_start(bqr_t[:, :], bqr_d[:, :])
        nc.sync.dma_start(bkr_t[:, :], bkr_d[:, :])
        nc.sync.dma_start(bvt_t[:, :], bvt_d[:, :])

        # ones: row 0 used as K=1 operand for Q/K/V bias outer products; row 64
        # (base partition 64, matching the Z row of out2) for the 1/Z broadcast.
        ones_t = consts.tile([65, SB], bf16)
        nc.any.memset(ones_t, 1.0)

        # diag mask: m128[tt, c] = 0 if c >= tt else NEG
        m128 = consts.tile([128, 128], f32)
        nc.any.memset(m128, 0.0)
        nc.gpsimd.affine_select(
            out=m128,
            in_=m128,
            compare_op=mybir.AluOpType.is_ge,
            fill=NEG,
            base=0,
            pattern=[[1, 128]],
            channel_multiplier=-1,
        )

        # ---- persistent K/V caches ----
        kt_all = cache.tile([128, 4, S], bf16)       # [d within pair chunk, pair, t]
        v_aug = cache.tile([128, 16, NHL, DH + 1], bf16)  # [t within chunk, tchunk, head, d|1]
        nc.any.memset(v_aug[:, :, :, DH : DH + 1], 1.0)

        for sb in range(NSB):
            s0 = sb * SB
            nkc = 4 * sb + 4

            xt_sb = xt_pool.tile([128, 8, SB], bf16, tag="xt")
            for ec in range(8):
                nc.sync.dma_start(xt_sb[:, ec, :], xt_d[ts(ec, 128), s0 : s0 + SB])

            # ---- Q/K projections: QT/KT [128 dims(pair), 512 s] per pair chunk ----
            qt_sb = qt_pool.tile([128, 4, SB], bf16, tag="qt")
            for dc in range(4):
                pq = ppool.tile([128, SB], f32, tag="pp")
                for ec in range(8):
                    nc.tensor.matmul(
                        pq, wq_t[:, ec, ts(dc, 128)], xt_sb[:, ec, :],
                        start=(ec == 0), stop=False,
                    )
                nc.tensor.matmul(
                    pq, bqr_t[0:1, ts(dc, 128)], ones_t[0:1, :],
                    start=False, stop=True,
                )
                nc.vector.tensor_copy(qt_sb[:, dc, :], pq)
            for dc in range(4):
                pk = ppool.tile([128, SB], f32, tag="pp")
                for ec in range(8):
                    nc.tensor.matmul(
                        pk, wk_t[:, ec, ts(dc, 128)], xt_sb[:, ec, :],
                        start=(ec == 0), stop=False,
                    )
                nc.tensor.matmul(
                    pk, bkr_t[0:1, ts(dc, 128)], ones_t[0:1, :],
                    start=False, stop=True,
                )
                nc.vector.tensor_copy(kt_all[:, dc, s0 : s0 + SB], pk)
            # ---- V projection (natural layout [t, d]) + bias via K=1 matmul ----
            for tsub in range(4):
                tcg = 4 * sb + tsub
                pv = ppool.tile([128, NHL, DH], f32, tag="pp")
                for ec in range(8):
                    nc.tensor.matmul(
                        pv, xt_sb[:, ec, ts(tsub, 128)], wv_t[:, ec, :],
                        start=(ec == 0), stop=False,
                    )
                nc.tensor.matmul(
                    pv, ones_t[0:1, 0:128], bvt_t, start=False, stop=True
                )
                nc.vector.tensor_copy(v_aug[:, tcg, :, 0:DH], pv[:, :, :])

            # ---- attention, per head-pair ----
            for p in range(4):
                out2 = [
                    pout2.tile([DH + 1, SB], f32, tag="po", name=f"out2_{hh}")
                    for hh in range(2)
                ]
                prev = None  # (exp tiles, col offset, key chunk)
                for kc in range(nkc):
                    j = kc - 4 * sb  # >= 0 on diagonal chunks
                    c0 = 128 * j if j > 0 else 0
                    cur_ps = []
                    for hh in range(2):
                        r0 = 64 * hh
                        ps_t = pscore.tile([128, SB], f32, tag="ps")
                        nc.tensor.matmul(
                            ps_t[:, c0:SB],
                            kt_all[r0 : r0 + 64, p, ts(kc, 128)],
                            qt_sb[r0 : r0 + 64, p, c0:SB],
                            start=True, stop=True,
                        )
                        cur_ps.append(ps_t)
                    if prev is not None:
                        pex, pc0, pkc = prev
                        for hh in range(2):
                            nc.tensor.matmul(
                                out2[hh][:, pc0:SB],
                                v_aug[:, pkc, 2 * p + hh, :],
                                pex[hh][:, pc0:SB],
                                start=(pkc == 0), stop=False,
                            )
                    cur_ex = []
                    for hh in range(2):
                        if j >= 0:
                            nc.vector.tensor_add(
                                cur_ps[hh][:, c0 : c0 + 128],
                                cur_ps[hh][:, c0 : c0 + 128],
                                m128,
                            )
                        ex = work.tile([128, SB], bf16, tag="expt", bufs=4)
                        nc.scalar.activation(
                            ex[:, c0:SB], cur_ps[hh][:, c0:SB], AF.Exp, scale=SCALE
                        )
                        cur_ex.append(ex)
                    prev = (cur_ex, c0, kc)
                # final attn@V for the last key chunk
                pex, pc0, pkc = prev
                for hh in range(2):
                    nc.tensor.matmul(
                        out2[hh][:, pc0:SB],
                        v_aug[:, pkc, 2 * p + hh, :],
                        pex[hh][:, pc0:SB],
                        start=(pkc == 0), stop=True,
                    )

                # ---- normalization: 1/Z, broadcast to 64 partitions, multiply ----
                ao_p = work.tile([128, SB], bf16, tag=f"ao{p}", bufs=1)
                for hh in range(2):
                    rz = work.tile([65, SB], f32, tag="rz", bufs=2)
                    nc.vector.reciprocal(
                        rz[DH : DH + 1, :], out2[hh][DH : DH + 1, :]
                    )
                    rz_bf = work.tile([65, SB], bf16, tag="rzbf", bufs=2)
                    nc.vector.tensor_copy(rz_bf[DH : DH + 1, :], rz[DH : DH + 1, :])
                    # broadcast 1/Z to 64 partitions: K=1 outer product
                    bc_ps = pscore.tile([128, SB], f32, tag="ps", name=f"bcps_{hh}")
                    nc.tensor.matmul(
                        bc_ps[0:64, :], ones_t[64:65, 0:64], rz_bf[DH : DH + 1, :],
                        start=True, stop=True,
                    )
                    bc_sb = work.tile([64, SB], f32, tag="bcsb", bufs=2)
                    nc.vector.tensor_copy(bc_sb, bc_ps[0:64, :])
                    if hh == 0:
                        nc.vector.tensor_mul(ao_p[0:64, :], out2[hh][0:DH, :], bc_sb)
                    else:
                        aotmp = work.tile([64, SB], bf16, tag="aotmp", bufs=2)
                        nc.vector.tensor_mul(aotmp, out2[hh][0:DH, :], bc_sb)
                        nc.gpsimd.dma_start(ao_p[64:128, :], aotmp)
                if p == 0:
                    ao_tiles = []
                ao_tiles.append(ao_p)

            # ---- out-projection: out[s, o] partial ----
            for sc in range(4):
                for oh in range(2):
                    po = ppool.tile([128, 512], f32, tag="pp")
                    for p in range(4):
                        nc.tensor.matmul(
                            po,
                            ao_tiles[p][:, ts(sc, 128)],
                            wo_t[:, p, ts(oh, 512)],
                            start=(p == 0), stop=(p == 3),
                        )
                    po_sb = work.tile([128, 512], f32, tag="posb", bufs=2)
                    nc.vector.tensor_copy(po_sb, po)
                    nc.sync.dma_start(
                        out_d[s0 + 128 * sc : s0 + 128 * (sc + 1), ts(oh, 512)], po_sb
                    )

    nc.compile()
    return nc


def _prepare_core_inputs(x, Wq, bq, Wk, bk, Wv, bv):
    """Build per-core input maps. Core c: b = c // 2, g = c % 2."""
    import ml_dtypes

    BF = ml_dtypes.bfloat16
    maps = []
    xt = [np.ascontiguousarray(x[b].T).astype(BF) for b in range(B)]
    wq_s, wk_s, wv_s, bq_s, bk_s, bv_s = [], [], [], [], [], []
    for g in range(HG):
        sl = slice(g * DG, (g + 1) * DG)
        wq_s.append(np.ascontiguousarray(Wq[sl, :].T).astype(BF))
        wk_s.append(np.ascontiguousarray(Wk[sl, :].T).astype(BF))
        wv_s.append(np.ascontiguousarray(Wv[sl, :].T).astype(BF))
        bq_s.append(bq[sl].reshape(1, DG).astype(BF))
        bk_s.append(bk[sl].reshape(1, DG).astype(BF))
        bv_s.append(bv[sl].reshape(1, DG).astype(BF))
    for c in range(B * HG):
        b, g = c // HG, c % HG
        maps.append({
            "xt": xt[b],
            "wqt": wq_s[g], "wkt": wk_s[g], "wvt": wv_s[g],
            "wot": None,  # filled by caller (needs Wo)
            "bqr": bq_s[g], "bkr": bk_s[g], "bvt": bv_s[g],
        })
    return maps


def kernel(x, Wq, bq, Wk, bk, Wv, bv, Wo, bo):
    from concourse.bass_utils import run_bass_kernel_spmd

    x = np.asarray(x, dtype=np.float32)
    Wq, bq = np.asarray(Wq, np.float32), np.asarray(bq, np.float32)
    Wk, bk = np.asarray(Wk, np.float32), np.asarray(bk, np.float32)
    Wv, bv = np.asarray(Wv, np.float32), np.asarray(bv, np.float32)
    Wo, bo = np.asarray(Wo, np.float32), np.asarray(bo, np.float32)

    if "nc" not in _CACHE:
        _CACHE["nc"] = _build_nc()
    nc = _CACHE["nc"]

    import ml_dtypes

    maps = _prepare_core_inputs(x, Wq, bq, Wk, bk, Wv, bv)
    wot = [
        np.ascontiguousarray(Wo.T[g * DG : (g + 1) * DG, :]).astype(ml_dtypes.bfloat16)
        for g in range(HG)
    ]
    for c in range(B * HG):
        maps[c]["wot"] = wot[c % HG]

    res = run_bass_kernel_spmd(nc, maps, list(range(B * HG)))
    _CACHE["last_results"] = res

    out = np.empty((B, S, D), dtype=np.float32)
    for b in range(B):
        out[b] = res.results[2 * b]["out"] + res.results[2 * b + 1]["out"] + bo
    return out



# revision 2
# speedup vs baseline: 1.6549x; 1.6549x over previous
"""Causal multi-head self-attention on 8 Trainium2 NeuronCores.

Sharding: 4 batches x 2 head-groups (8 heads each). Core c = (b, g) with
b = c // 2, g = c % 2. Each core computes QKV projections for its weight
row-slice, attention for its 8 heads, and a partial out-projection
(Megatron row-parallel). Host sums the two partials per batch and adds
bo + bv @ Wo.T (the V-bias folds out of attention exactly: softmax rows
sum to 1).

All shapes hardcoded for x [4, 2048, 1024], 16 heads, head_dim 64, fp32.
"""

import sys
import numpy as np

if "/opt/trn_rl_repo" not in sys.path:
    sys.path.insert(0, "/opt/trn_rl_repo")

B = 4
S = 2048
D = 1024
HG = 2            # head groups (cores per batch)
NHL = 8           # heads per core
DH = 64
DG = NHL * DH     # 512 feature dims per core
SB = 512          # s-block
NSB = S // SB     # 4
SCALE = 0.125     # 1/sqrt(64)

_CACHE = {}


def _build_nc():
    import concourse.bass as bass
    import concourse.bacc as bacc
    import concourse.tile as tile
    from concourse import mybir
    from contextlib import ExitStack

    f32 = mybir.dt.float32
    bf16 = mybir.dt.bfloat16
    AF = mybir.ActivationFunctionType
    ts = bass.ts

    nc = bacc.Bacc(None, target_bir_lowering=False)

    xt_d = nc.dram_tensor("xt", [D, S], bf16, kind="ExternalInput")
    wqt_d = nc.dram_tensor("wqt", [D, DG], bf16, kind="ExternalInput")
    wkt_d = nc.dram_tensor("wkt", [D, DG], bf16, kind="ExternalInput")
    wvt_d = nc.dram_tensor("wvt", [D, DG], bf16, kind="ExternalInput")
    wot_d = nc.dram_tensor("wot", [DG, D], bf16, kind="ExternalInput")
    bqc_d = nc.dram_tensor("bqc", [128, 4], f32, kind="ExternalInput")
    bkc_d = nc.dram_tensor("bkc", [128, 4], f32, kind="ExternalInput")
    out_d = nc.dram_tensor("out", [S, D], bf16, kind="ExternalOutput")

    with tile.TileContext(nc) as tc, ExitStack() as ctx:
        consts = ctx.enter_context(tc.tile_pool(name="consts", bufs=1))
        cache = ctx.enter_context(tc.tile_pool(name="cache", bufs=1))
        xt_pool = ctx.enter_context(tc.tile_pool(name="xtp", bufs=1))
        qt_pool = ctx.enter_context(tc.tile_pool(name="qtp", bufs=1))
        work = ctx.enter_context(tc.tile_pool(name="work", bufs=1))
        ppool = ctx.enter_context(tc.tile_pool(name="pp", bufs=2, space="PSUM"))
        pscore = ctx.enter_context(tc.tile_pool(name="ps", bufs=2, space="PSUM"))
        pout2 = ctx.enter_context(tc.tile_pool(name="po", bufs=2, space="PSUM"))

        # ---- constants / weights in SBUF ----
        wq_t = consts.tile([128, 8, DG], bf16)
        wk_t = consts.tile([128, 8, DG], bf16)
        wv_t = consts.tile([128, 8, DG], bf16)
        wo_t = consts.tile([128, 4, D], bf16)
        for ec in range(8):
            nc.sync.dma_start(wq_t[:, ec, :], wqt_d[ts(ec, 128), :])
            nc.sync.dma_start(wk_t[:, ec, :], wkt_d[ts(ec, 128), :])
            nc.sync.dma_start(wv_t[:, ec, :], wvt_d[ts(ec, 128), :])
        for p in range(4):
            nc.sync.dma_start(wo_t[:, p, :], wot_d[ts(p, 128), :])
        bqc_t = consts.tile([128, 4], f32)
        bkc_t = consts.tile([128, 4], f32)
        nc.sync.dma_start(bqc_t[:, :], bqc_d[:, :])
        nc.sync.dma_start(bkc_t[:, :], bkc_d[:, :])

        # row 64: K=1 operand for the 1/Z partition-broadcast matmul
        ones_t = consts.tile([65, 64], bf16)
        nc.any.memset(ones_t, 1.0)

        # ---- persistent K/V caches ----
        kt_all = cache.tile([128, 4, S], bf16)       # [d within pair chunk, pair, t]
        v_aug = cache.tile([128, 16, NHL, DH + 1], bf16)  # [t within chunk, tchunk, head, d|1]
        nc.any.memset(v_aug[:, :, :, DH : DH + 1], 1.0)

        for sb in range(NSB):
            s0 = sb * SB
            nkc = 4 * sb + 4

            xt_sb = xt_pool.tile([128, 8, SB], bf16, tag="xt")
            for ec in range(8):
                nc.sync.dma_start(xt_sb[:, ec, :], xt_d[ts(ec, 128), s0 : s0 + SB])

            # ---- Q/K projections: QT/KT [128 dims(pair), 512 s] per pair chunk ----
            qt_sb = qt_pool.tile([128, 4, SB], bf16, tag="qt")
            for dc in range(4):
                pq = ppool.tile([128, SB], f32, tag="pp")
                for ec in range(8):
                    nc.tensor.matmul(
                        pq, wq_t[:, ec, ts(dc, 128)], xt_sb[:, ec, :],
                        start=(ec == 0), stop=(ec == 7),
                    )
                nc.vector.tensor_scalar_add(qt_sb[:, dc, :], pq, bqc_t[:, dc : dc + 1])
            for dc in range(4):
                pk = ppool.tile([128, SB], f32, tag="pp")
                for ec in range(8):
                    nc.tensor.matmul(
                        pk, wk_t[:, ec, ts(dc, 128)], xt_sb[:, ec, :],
                        start=(ec == 0), stop=(ec == 7),
                    )
                nc.vector.tensor_scalar_add(
                    kt_all[:, dc, s0 : s0 + SB], pk, bkc_t[:, dc : dc + 1]
                )
            # ---- V projection (natural layout [t, d]); bias folded on host ----
            for tsub in range(4):
                tcg = 4 * sb + tsub
                pv = ppool.tile([128, NHL, DH], f32, tag="pp")
                for ec in range(8):
                    nc.tensor.matmul(
                        pv, xt_sb[:, ec, ts(tsub, 128)], wv_t[:, ec, :],
                        start=(ec == 0), stop=(ec == 7),
                    )
                nc.vector.tensor_copy(v_aug[:, tcg, :, 0:DH], pv[:, :, :])

            # ---- attention, per head-pair ----
            for p in range(4):
                out2 = [
                    pout2.tile([DH + 1, SB], f32, tag="po", name=f"out2_{hh}")
                    for hh in range(2)
                ]
                prev = None  # (exp tile, col offset, key chunk)
                for kc in range(nkc):
                    j = kc - 4 * sb  # >= 0 on diagonal chunks
                    c0 = 128 * j if j > 0 else 0
                    ps_t = pscore.tile([128, 2, SB], f32, tag="ps")
                    for hh in range(2):
                        r0 = 64 * hh
                        nc.tensor.matmul(
                            ps_t[:, hh, c0:SB],
                            kt_all[r0 : r0 + 64, p, ts(kc, 128)],
                            qt_sb[r0 : r0 + 64, p, c0:SB],
                            start=True, stop=True,
                        )
                    if prev is not None:
                        pex, pc0, pkc = prev
                        for hh in range(2):
                            nc.tensor.matmul(
                                out2[hh][:, pc0:SB],
                                v_aug[:, pkc, 2 * p + hh, :],
                                pex[:, hh, pc0:SB],
                                start=(pkc == 0), stop=False,
                            )
                    ex = work.tile([128, 2, SB], bf16, tag="expt", bufs=4)
                    nc.scalar.activation(
                        ex[:, :, c0:SB], ps_t[:, :, c0:SB], AF.Exp, scale=SCALE
                    )
                    if j >= 0:
                        # causal mask: zero ex[k, hh, q] where q < k within the
                        # 128x128 diagonal block (iota = col - chan, keep >= 0)
                        nc.gpsimd.affine_select(
                            out=ex[:, :, c0 : c0 + 128],
                            in_=ex[:, :, c0 : c0 + 128],
                            compare_op=mybir.AluOpType.is_ge,
                            fill=0.0,
                            base=0,
                            pattern=[[0, 2], [1, 128]],
                            channel_multiplier=-1,
                        )
                    prev = (ex, c0, kc)
                # final attn@V for the last key chunk
                pex, pc0, pkc = prev
                for hh in range(2):
                    nc.tensor.matmul(
                        out2[hh][:, pc0:SB],
                        v_aug[:, pkc, 2 * p + hh, :],
                        pex[:, hh, pc0:SB],
                        start=(pkc == 0), stop=True,
                    )

                # ---- normalization: broadcast Z, reciprocal, multiply ----
                ao_p = work.tile([128, SB], bf16, tag=f"ao{p}", bufs=1)
                for hh in range(2):
                    z_bf = work.tile([65, SB], bf16, tag="zbf", bufs=2)
                    nc.vector.tensor_copy(z_bf[64:65, :], out2[hh][DH : DH + 1, :])
                    bc_ps = ppool.tile([64, SB], f32, tag="pp", name=f"bcps_{hh}")
                    nc.tensor.matmul(
                        bc_ps[0:64, :], ones_t[64:65, 0:64], z_bf[64:65, :],
                        start=True, stop=True,
                    )
                    rbc = work.tile([64, SB], f32, tag="rbc", bufs=2)
                    nc.vector.reciprocal_approx_fast(rbc, bc_ps[0:64, :])
                    if hh == 0:
                        nc.vector.tensor_mul(ao_p[0:64, :], out2[hh][0:DH, :], rbc)
                    else:
                        aotmp = work.tile([64, SB], bf16, tag="aotmp", bufs=2)
                        nc.vector.tensor_mul(aotmp, out2[hh][0:DH, :], rbc)
                        nc.gpsimd.dma_start(ao_p[64:128, :], aotmp)
                if p == 0:
                    ao_tiles = []
                ao_tiles.append(ao_p)

            # ---- out-projection: out[s, o] partial ----
            for sc in range(4):
                for oh in range(2):
                    po = ppool.tile([128, 512], f32, tag="pp")
                    for p in range(4):
                        nc.tensor.matmul(
                            po,
                            ao_tiles[p][:, ts(sc, 128)],
                            wo_t[:, p, ts(oh, 512)],
                            start=(p == 0), stop=(p == 3),
                        )
                    po_sb = work.tile([128, 512], bf16, tag="posb", bufs=2)
                    nc.vector.tensor_copy(po_sb, po)
                    nc.sync.dma_start(
                        out_d[s0 + 128 * sc : s0 + 128 * (sc + 1), ts(oh, 512)], po_sb
                    )

    nc.compile()
    return nc


def _prepare_core_inputs(x, Wq, bq, Wk, bk, Wv):
    """Build per-core input maps. Core c: b = c // 2, g = c % 2."""
    import ml_dtypes

    BF = ml_dtypes.bfloat16
    maps = []
    xt = [np.ascontiguousarray(x[b].T).astype(BF) for b in range(B)]
    wq_s, wk_s, wv_s, bq_s, bk_s = [], [], [], [], []
    for g in range(HG):
        sl = slice(g * DG, (g + 1) * DG)
        wq_s.append(np.ascontiguousarray(Wq[sl, :].T).astype(BF))
        wk_s.append(np.ascontiguousarray(Wk[sl, :].T).astype(BF))
        wv_s.append(np.ascontiguousarray(Wv[sl, :].T).astype(BF))
        # per-dim bias columns: [128, 4] = bias[dc*128 + r] at [r, dc]
        bq_s.append(np.ascontiguousarray(bq[sl].reshape(4, 128).T).astype(np.float32))
        bk_s.append(np.ascontiguousarray(bk[sl].reshape(4, 128).T).astype(np.float32))
    for c in range(B * HG):
        b, g = c // HG, c % HG
        maps.append({
            "xt": xt[b],
            "wqt": wq_s[g], "wkt": wk_s[g], "wvt": wv_s[g],
            "wot": None,  # filled by caller (needs Wo)
            "bqc": bq_s[g], "bkc": bk_s[g],
        })
    return maps


def kernel(x, Wq, bq, Wk, bk, Wv, bv, Wo, bo):
    from concourse.bass_utils import run_bass_kernel_spmd

    x = np.asarray(x, dtype=np.float32)
    Wq, bq = np.asarray(Wq, np.float32), np.asarray(bq, np.float32)
    Wk, bk = np.asarray(Wk, np.float32), np.asarray(bk, np.float32)
    Wv, bv = np.asarray(Wv, np.float32), np.asarray(bv, np.float32)
    Wo, bo = np.asarray(Wo, np.float32), np.asarray(bo, np.float32)

    if "nc" not in _CACHE:
        _CACHE["nc"] = _build_nc()
    nc = _CACHE["nc"]

    import ml_dtypes

    maps = _prepare_core_inputs(x, Wq, bq, Wk, bk, Wv)
    wot = [
        np.ascontiguousarray(Wo.T[g * DG : (g + 1) * DG, :]).astype(ml_dtypes.bfloat16)
        for g in range(HG)
    ]
    for c in range(B * HG):
        maps[c]["wot"] = wot[c % HG]

    res = run_bass_kernel_spmd(nc, maps, list(range(B * HG)))
    _CACHE["last_results"] = res

    # V-bias folds out of attention exactly (softmax rows sum to 1)
    bo_eff = bo + bv @ Wo.T

    out = np.empty((B, S, D), dtype=np.float32)
    for b in range(B):
        out[b] = (
            res.results[2 * b]["out"].astype(np.float32)
            + res.results[2 * b + 1]["out"].astype(np.float32)
            + bo_eff
        )
    return out


# revision 3
# speedup vs baseline: 1.9205x; 1.1605x over previous
"""Causal multi-head self-attention on 8 Trainium2 NeuronCores.

Sharding: 4 batches x 2 head-groups (8 heads each). Core c = (b, g) with
b = c // 2, g = c % 2. Each core computes QKV projections for its weight
row-slice, attention for its 8 heads, and a partial out-projection
(Megatron row-parallel). Host sums the two partials per batch and adds
bo + bv @ Wo.T (the V-bias folds out of attention exactly: softmax rows
sum to 1).

The instruction stream software-pipelines phases: QKV-projection matmul
groups for s-block sb+1 and out-projection groups for sb-1 are emitted
between attention chunks of sb, so the tensor engine fills the gaps
where attention is paced by the scalar-engine exp.

All shapes hardcoded for x [4, 2048, 1024], 16 heads, head_dim 64, fp32.
"""

import sys
import numpy as np

if "/opt/trn_rl_repo" not in sys.path:
    sys.path.insert(0, "/opt/trn_rl_repo")

B = 4
S = 2048
D = 1024
HG = 2            # head groups (cores per batch)
NHL = 8           # heads per core
DH = 64
DG = NHL * DH     # 512 feature dims per core
SB = 512          # s-block
NSB = S // SB     # 4
SCALE = 0.125     # 1/sqrt(64)

_CACHE = {}


def _build_nc():
    import concourse.bass as bass
    import concourse.bacc as bacc
    import concourse.tile as tile
    from concourse import mybir
    from contextlib import ExitStack

    f32 = mybir.dt.float32
    bf16 = mybir.dt.bfloat16
    AF = mybir.ActivationFunctionType
    ts = bass.ts

    nc = bacc.Bacc(None, target_bir_lowering=False)

    xt_d = nc.dram_tensor("xt", [D, S], bf16, kind="ExternalInput")
    wqt_d = nc.dram_tensor("wqt", [D, DG], bf16, kind="ExternalInput")
    wkt_d = nc.dram_tensor("wkt", [D, DG], bf16, kind="ExternalInput")
    wvt_d = nc.dram_tensor("wvt", [D, DG], bf16, kind="ExternalInput")
    wot_d = nc.dram_tensor("wot", [DG, D], bf16, kind="ExternalInput")
    bqc_d = nc.dram_tensor("bqc", [128, 4], f32, kind="ExternalInput")
    bkc_d = nc.dram_tensor("bkc", [128, 4], f32, kind="ExternalInput")
    out_d = nc.dram_tensor("out", [S, D], bf16, kind="ExternalOutput")

    with tile.TileContext(nc) as tc, ExitStack() as ctx:
        consts = ctx.enter_context(tc.tile_pool(name="consts", bufs=1))
        cache = ctx.enter_context(tc.tile_pool(name="cache", bufs=1))
        xt_pool = ctx.enter_context(tc.tile_pool(name="xtp", bufs=2))
        qt_pool = ctx.enter_context(tc.tile_pool(name="qtp", bufs=2))
        work = ctx.enter_context(tc.tile_pool(name="work", bufs=1))
        ppool = ctx.enter_context(tc.tile_pool(name="pp", bufs=2, space="PSUM"))
        pscore = ctx.enter_context(tc.tile_pool(name="ps", bufs=2, space="PSUM"))
        pout2 = ctx.enter_context(tc.tile_pool(name="po", bufs=2, space="PSUM"))

        # row 64: K=1 operand for the 1/Z partition-broadcast matmul.
        # Memset'd (no DMA dep) so it also feeds the HAM warmup matmuls.
        ones_t = consts.tile([65, 64], bf16)
        nc.any.memset(ones_t, 1.0)

        # ---- HAM warmup: keep PE busy ~4us while input DMAs land ----
        for w in range(40):
            pwarm = ppool.tile([64, 64], f32, tag="pp", name=f"pwarm_{w}")
            nc.tensor.matmul(
                pwarm[0:64, 0:64], ones_t[0:64, 0:64], ones_t[0:64, 0:64],
                start=True, stop=True,
            )

        # ---- weights / constants in SBUF (ordered so proj(0) starts early) ----
        wq_t = consts.tile([128, 8, DG], bf16)
        wk_t = consts.tile([128, 8, DG], bf16)
        wv_t = consts.tile([128, 8, DG], bf16)
        wo_t = consts.tile([128, 4, D], bf16)
        bqc_t = consts.tile([128, 4], f32)
        bkc_t = consts.tile([128, 4], f32)

        xt_tiles = {}

        def emit_xt_load(sb):
            s0 = sb * SB
            xt_sb = xt_pool.tile([128, 8, SB], bf16, tag="xt", name=f"xt_{sb}")
            for ec in range(8):
                q = nc.sync if ec % 2 == 0 else nc.gpsimd
                q.dma_start(xt_sb[:, ec, :], xt_d[ts(ec, 128), s0 : s0 + SB])
            xt_tiles[sb] = xt_sb

        emit_xt_load(0)
        for ec in range(8):
            nc.sync.dma_start(wq_t[:, ec, :], wqt_d[ts(ec, 128), :])
            nc.gpsimd.dma_start(wk_t[:, ec, :], wkt_d[ts(ec, 128), :])
            nc.sync.dma_start(wv_t[:, ec, :], wvt_d[ts(ec, 128), :])
        nc.sync.dma_start(bqc_t[:, :], bqc_d[:, :])
        nc.sync.dma_start(bkc_t[:, :], bkc_d[:, :])
        for p in range(4):
            nc.gpsimd.dma_start(wo_t[:, p, :], wot_d[ts(p, 128), :])

        # ---- persistent K/V caches ----
        kt_all = cache.tile([128, 4, S], bf16)       # [d within pair chunk, pair, t]
        v_aug = cache.tile([128, 16, NHL, DH + 1], bf16)  # [t within chunk, tchunk, head, d|1]
        nc.any.memset(v_aug[:, :, :, DH : DH + 1], 1.0)

        qt_tiles = {}
        ao_tiles = {}

        # ---- work-item emitters (each is one PSUM-group of tensor work) ----
        def emit_proj_q(sb, dc):
            xt_sb = xt_tiles[sb]
            if dc == 0:
                qt_tiles[sb] = qt_pool.tile(
                    [128, 4, SB], bf16, tag="qt", name=f"qt_{sb}"
                )
            qt_sb = qt_tiles[sb]
            pq = ppool.tile([128, SB], f32, tag="pp", name=f"pq_{sb}_{dc}")
            for ec in range(8):
                nc.tensor.matmul(
                    pq, wq_t[:, ec, ts(dc, 128)], xt_sb[:, ec, :],
                    start=(ec == 0), stop=(ec == 7),
                )
            nc.vector.tensor_scalar_add(qt_sb[:, dc, :], pq, bqc_t[:, dc : dc + 1])

        def emit_proj_k(sb, dc):
            xt_sb = xt_tiles[sb]
            s0 = sb * SB
            pk = ppool.tile([128, SB], f32, tag="pp", name=f"pk_{sb}_{dc}")
            for ec in range(8):
                nc.tensor.matmul(
                    pk, wk_t[:, ec, ts(dc, 128)], xt_sb[:, ec, :],
                    start=(ec == 0), stop=(ec == 7),
                )
            nc.vector.tensor_scalar_add(
                kt_all[:, dc, s0 : s0 + SB], pk, bkc_t[:, dc : dc + 1]
            )

        def emit_proj_v(sb, tsub):
            xt_sb = xt_tiles[sb]
            tcg = 4 * sb + tsub
            pv = ppool.tile([128, NHL, DH], f32, tag="pp", name=f"pv_{sb}_{tsub}")
            for ec in range(8):
                nc.tensor.matmul(
                    pv, xt_sb[:, ec, ts(tsub, 128)], wv_t[:, ec, :],
                    start=(ec == 0), stop=(ec == 7),
                )
            nc.vector.tensor_copy(v_aug[:, tcg, :, 0:DH], pv[:, :, :])

        def emit_outproj(sb, sc, oh):
            s0 = sb * SB
            ao = ao_tiles[sb]
            po = ppool.tile([128, 512], f32, tag="pp", name=f"pop_{sb}_{sc}_{oh}")
            for p in range(4):
                nc.tensor.matmul(
                    po,
                    ao[p][:, ts(sc, 128)],
                    wo_t[:, p, ts(oh, 512)],
                    start=(p == 0), stop=(p == 3),
                )
            po_sb = work.tile([128, 512], bf16, tag="posb", bufs=2)
            nc.vector.tensor_copy(po_sb, po)
            nc.sync.dma_start(
                out_d[s0 + 128 * sc : s0 + 128 * (sc + 1), ts(oh, 512)], po_sb
            )

        def proj_items(sb):
            items = []
            for dc in range(4):
                items.append(lambda sb=sb, dc=dc: emit_proj_q(sb, dc))
            for dc in range(4):
                items.append(lambda sb=sb, dc=dc: emit_proj_k(sb, dc))
            for tsub in range(4):
                items.append(lambda sb=sb, tsub=tsub: emit_proj_v(sb, tsub))
            return items

        def outproj_items(sb):
            return [
                lambda sb=sb, sc=sc, oh=oh: emit_outproj(sb, sc, oh)
                for sc in range(4)
                for oh in range(2)
            ]

        # proj(0) runs standalone up front
        for it in proj_items(0):
            it()

        for sb in range(NSB):
            s0 = sb * SB
            nkc = 4 * sb + 4
            qt_sb = qt_tiles[sb]

            if sb < NSB - 1:
                emit_xt_load(sb + 1)

            # filler: outproj(sb-1) first (frees ao tiles), then proj(sb+1)
            fill = []
            if sb > 0:
                fill += outproj_items(sb - 1)
            if sb < NSB - 1:
                fill += proj_items(sb + 1)
            total_chunks = 4 * nkc
            stride = max(1, total_chunks // max(1, len(fill)))
            chunk_i = 0

            ao_tiles[sb] = []
            for p in range(4):
                out2 = [
                    pout2.tile([DH + 1, SB], f32, tag="po", name=f"out2_{hh}")
                    for hh in range(2)
                ]
                prev = None  # (exp tile, col offset, key chunk)
                for kc in range(nkc):
                    j = kc - 4 * sb  # >= 0 on diagonal chunks
                    c0 = 128 * j if j > 0 else 0
                    ps_t = pscore.tile([128, 2, SB], f32, tag="ps")
                    for hh in range(2):
                        r0 = 64 * hh
                        nc.tensor.matmul(
                            ps_t[:, hh, c0:SB],
                            kt_all[r0 : r0 + 64, p, ts(kc, 128)],
                            qt_sb[r0 : r0 + 64, p, c0:SB],
                            start=True, stop=True,
                        )
                    if prev is not None:
                        pex, pc0, pkc = prev
                        for hh in range(2):
                            nc.tensor.matmul(
                                out2[hh][:, pc0:SB],
                                v_aug[:, pkc, 2 * p + hh, :],
                                pex[:, hh, pc0:SB],
                                start=(pkc == 0), stop=False,
                            )
                    ex = work.tile([128, 2, SB], bf16, tag="expt", bufs=4)
                    nc.scalar.activation(
                        ex[:, :, c0:SB], ps_t[:, :, c0:SB], AF.Exp, scale=SCALE
                    )
                    if j >= 0:
                        # causal mask: zero ex[k, hh, q] where q < k within the
                        # 128x128 diagonal block (iota = col - chan, keep >= 0)
                        nc.gpsimd.affine_select(
                            out=ex[:, :, c0 : c0 + 128],
                            in_=ex[:, :, c0 : c0 + 128],
                            compare_op=mybir.AluOpType.is_ge,
                            fill=0.0,
                            base=0,
                            pattern=[[0, 2], [1, 128]],
                            channel_multiplier=-1,
                        )
                    prev = (ex, c0, kc)
                    chunk_i += 1
                    if fill and chunk_i % stride == 0:
                        fill.pop(0)()
                # final attn@V for the last key chunk
                pex, pc0, pkc = prev
                for hh in range(2):
                    nc.tensor.matmul(
                        out2[hh][:, pc0:SB],
                        v_aug[:, pkc, 2 * p + hh, :],
                        pex[:, hh, pc0:SB],
                        start=(pkc == 0), stop=True,
                    )

                # ---- normalization: broadcast Z, reciprocal, multiply ----
                ao_p = work.tile([128, SB], bf16, tag=f"ao{p}", bufs=2)
                for hh in range(2):
                    z_bf = work.tile([65, SB], bf16, tag="zbf", bufs=2)
                    nc.vector.tensor_copy(z_bf[64:65, :], out2[hh][DH : DH + 1, :])
                    bc_ps = ppool.tile([64, SB], f32, tag="pp", name=f"bcps_{hh}")
                    nc.tensor.matmul(
                        bc_ps[0:64, :], ones_t[64:65, 0:64], z_bf[64:65, :],
                        start=True, stop=True,
                    )
                    rbc = work.tile([64, SB], f32, tag="rbc", bufs=2)
                    nc.vector.reciprocal_approx_fast(rbc, bc_ps[0:64, :])
                    if hh == 0:
                        nc.vector.tensor_mul(ao_p[0:64, :], out2[hh][0:DH, :], rbc)
                    else:
                        aotmp = work.tile([64, SB], bf16, tag="aotmp", bufs=2)
                        nc.vector.tensor_mul(aotmp, out2[hh][0:DH, :], rbc)
                        nc.gpsimd.dma_start(ao_p[64:128, :], aotmp)
                ao_tiles[sb].append(ao_p)

            # drain any leftover filler
            for it in fill:
                it()

        for it in outproj_items(NSB - 1):
            it()

    nc.compile()
    return nc


def _prepare_core_inputs(x, Wq, bq, Wk, bk, Wv):
    """Build per-core input maps. Core c: b = c // 2, g = c % 2."""
    import ml_dtypes

    BF = ml_dtypes.bfloat16
    maps = []
    xt = [np.ascontiguousarray(x[b].T).astype(BF) for b in range(B)]
    wq_s, wk_s, wv_s, bq_s, bk_s = [], [], [], [], []
    for g in range(HG):
        sl = slice(g * DG, (g + 1) * DG)
        wq_s.append(np.ascontiguousarray(Wq[sl, :].T).astype(BF))
        wk_s.append(np.ascontiguousarray(Wk[sl, :].T).astype(BF))
        wv_s.append(np.ascontiguousarray(Wv[sl, :].T).astype(BF))
        # per-dim bias columns: [128, 4] = bias[dc*128 + r] at [r, dc]
        bq_s.append(np.ascontiguousarray(bq[sl].reshape(4, 128).T).astype(np.float32))
        bk_s.append(np.ascontiguousarray(bk[sl].reshape(4, 128).T).astype(np.float32))
    for c in range(B * HG):
        b, g = c // HG, c % HG
        maps.append({
            "xt": xt[b],
            "wqt": wq_s[g], "wkt": wk_s[g], "wvt": wv_s[g],
            "wot": None,  # filled by caller (needs Wo)
            "bqc": bq_s[g], "bkc": bk_s[g],
        })
    return maps


def kernel(x, Wq, bq, Wk, bk, Wv, bv, Wo, bo):
    from concourse.bass_utils import run_bass_kernel_spmd

    x = np.asarray(x, dtype=np.float32)
    Wq, bq = np.asarray(Wq, np.float32), np.asarray(bq, np.float32)
    Wk, bk = np.asarray(Wk, np.float32), np.asarray(bk, np.float32)
    Wv, bv = np.asarray(Wv, np.float32), np.asarray(bv, np.float32)
    Wo, bo = np.asarray(Wo, np.float32), np.asarray(bo, np.float32)

    if "nc" not in _CACHE:
        _CACHE["nc"] = _build_nc()
    nc = _CACHE["nc"]

    import ml_dtypes

    maps = _prepare_core_inputs(x, Wq, bq, Wk, bk, Wv)
    wot = [
        np.ascontiguousarray(Wo.T[g * DG : (g + 1) * DG, :]).astype(ml_dtypes.bfloat16)
        for g in range(HG)
    ]
    for c in range(B * HG):
        maps[c]["wot"] = wot[c % HG]

    res = run_bass_kernel_spmd(nc, maps, list(range(B * HG)))
    _CACHE["last_results"] = res

    # V-bias folds out of attention exactly (softmax rows sum to 1)
    bo_eff = bo + bv @ Wo.T

    out = np.empty((B, S, D), dtype=np.float32)
    for b in range(B):
        out[b] = (
            res.results[2 * b]["out"].astype(np.float32)
            + res.results[2 * b + 1]["out"].astype(np.float32)
            + bo_eff
        )
    return out


# revision 5
# speedup vs baseline: 1.9662x; 1.0238x over previous
"""Causal multi-head self-attention on 8 Trainium2 NeuronCores.

Sharding: 4 batches x 2 head-groups (8 heads each). Core c = (b, g) with
b = c // 2, g = c % 2. Each core computes QKV projections for its weight
row-slice, attention for its 8 heads, and a partial out-projection
(Megatron row-parallel). Host sums the two partials per batch and adds
bo + bv @ Wo.T (the V-bias folds out of attention exactly: softmax rows
sum to 1).

The instruction stream software-pipelines phases: QKV-projection matmul
groups for later s-blocks and out-projection groups for earlier s-blocks
are emitted between attention chunks, placed so that tensor-engine work
fills the phases where attention is paced by the scalar-engine exp (the
last s-block's attention has the most exp work, so all out-projections
land there).

All shapes hardcoded for x [4, 2048, 1024], 16 heads, head_dim 64, fp32.
"""

import sys
import numpy as np

if "/opt/trn_rl_repo" not in sys.path:
    sys.path.insert(0, "/opt/trn_rl_repo")

B = 4
S = 2048
D = 1024
HG = 2            # head groups (cores per batch)
NHL = 8           # heads per core
DH = 64
DG = NHL * DH     # 512 feature dims per core
SB = 512          # s-block
NSB = S // SB     # 4
SCALE = 0.125     # 1/sqrt(64)

_CACHE = {}


def _build_nc():
    import concourse.bass as bass
    import concourse.bacc as bacc
    import concourse.tile as tile
    from concourse import mybir
    from contextlib import ExitStack

    f32 = mybir.dt.float32
    bf16 = mybir.dt.bfloat16
    AF = mybir.ActivationFunctionType
    ts = bass.ts

    nc = bacc.Bacc(None, target_bir_lowering=False)

    xt_d = nc.dram_tensor("xt", [D, S], bf16, kind="ExternalInput")
    wqt_d = nc.dram_tensor("wqt", [D, DG], bf16, kind="ExternalInput")
    wkt_d = nc.dram_tensor("wkt", [D, DG], bf16, kind="ExternalInput")
    wvt_d = nc.dram_tensor("wvt", [D, DG], bf16, kind="ExternalInput")
    wot_d = nc.dram_tensor("wot", [DG, D], bf16, kind="ExternalInput")
    bqc_d = nc.dram_tensor("bqc", [128, 4], f32, kind="ExternalInput")
    bkc_d = nc.dram_tensor("bkc", [128, 4], f32, kind="ExternalInput")
    out_d = nc.dram_tensor("out", [S, D], bf16, kind="ExternalOutput")

    with tile.TileContext(nc) as tc, ExitStack() as ctx:
        consts = ctx.enter_context(tc.tile_pool(name="consts", bufs=1))
        cache = ctx.enter_context(tc.tile_pool(name="cache", bufs=1))
        xt_pool = ctx.enter_context(tc.tile_pool(name="xtp", bufs=2))
        qt_pool = ctx.enter_context(tc.tile_pool(name="qtp", bufs=3))
        work = ctx.enter_context(tc.tile_pool(name="work", bufs=1))
        ppool = ctx.enter_context(tc.tile_pool(name="pp", bufs=2, space="PSUM"))
        pscore = ctx.enter_context(tc.tile_pool(name="ps", bufs=2, space="PSUM"))
        pout2 = ctx.enter_context(tc.tile_pool(name="po", bufs=2, space="PSUM"))

        ones_t = consts.tile([65, 64], bf16)
        nc.any.memset(ones_t, 1.0)

        # ---- HAM warmup: keep PE busy ~4us while input DMAs land ----
        for w in range(40):
            pwarm = ppool.tile([64, 64], f32, tag="pp", name=f"pwarm_{w}")
            nc.tensor.matmul(
                pwarm[0:64, 0:64], ones_t[0:64, 0:64], ones_t[0:64, 0:64],
                start=True, stop=True,
            )

        # ---- weights / constants in SBUF (ordered so proj(0) starts early) ----
        wq_t = consts.tile([128, 8, DG], bf16)
        wk_t = consts.tile([128, 8, DG], bf16)
        wv_t = consts.tile([128, 8, DG], bf16)
        wo_t = consts.tile([128, 4, D], bf16)
        bqc_t = consts.tile([128, 4], f32)
        bkc_t = consts.tile([128, 4], f32)

        xt_tiles = {}

        def emit_xt_load(sb):
            s0 = sb * SB
            xt_sb = xt_pool.tile([128, 8, SB], bf16, tag="xt", name=f"xt_{sb}")
            for ec in range(8):
                q = nc.sync if ec % 2 == 0 else nc.gpsimd
                q.dma_start(xt_sb[:, ec, :], xt_d[ts(ec, 128), s0 : s0 + SB])
            xt_tiles[sb] = xt_sb

        emit_xt_load(0)
        for ec in range(8):
            nc.sync.dma_start(wq_t[:, ec, :], wqt_d[ts(ec, 128), :])
            nc.gpsimd.dma_start(wk_t[:, ec, :], wkt_d[ts(ec, 128), :])
            nc.sync.dma_start(wv_t[:, ec, :], wvt_d[ts(ec, 128), :])
        nc.sync.dma_start(bqc_t[:, :], bqc_d[:, :])
        nc.sync.dma_start(bkc_t[:, :], bkc_d[:, :])
        for p in range(4):
            nc.gpsimd.dma_start(wo_t[:, p, :], wot_d[ts(p, 128), :])

        # ---- persistent K/V caches ----
        kt_all = cache.tile([128, 4, S], bf16)       # [d within pair chunk, pair, t]
        v_aug = cache.tile([128, 16, NHL, DH + 1], bf16)  # [t within chunk, tchunk, head, d|1]
        nc.any.memset(v_aug[:, :, :, DH : DH + 1], 1.0)

        qt_tiles = {}
        ao_tiles = {}

        # ---- work-item emitters (each is one PSUM-group of tensor work) ----
        def emit_proj_q(sb, dc):
            xt_sb = xt_tiles[sb]
            if dc == 0:
                qt_tiles[sb] = qt_pool.tile(
                    [128, 4, SB], bf16, tag="qt", name=f"qt_{sb}"
                )
            qt_sb = qt_tiles[sb]
            pq = ppool.tile([128, SB], f32, tag="pp", name=f"pq_{sb}_{dc}")
            for ec in range(8):
                nc.tensor.matmul(
                    pq, wq_t[:, ec, ts(dc, 128)], xt_sb[:, ec, :],
                    start=(ec == 0), stop=(ec == 7),
                )
            nc.vector.tensor_scalar_add(qt_sb[:, dc, :], pq, bqc_t[:, dc : dc + 1])

        def emit_proj_k(sb, dc):
            xt_sb = xt_tiles[sb]
            s0 = sb * SB
            pk = ppool.tile([128, SB], f32, tag="pp", name=f"pk_{sb}_{dc}")
            for ec in range(8):
                nc.tensor.matmul(
                    pk, wk_t[:, ec, ts(dc, 128)], xt_sb[:, ec, :],
                    start=(ec == 0), stop=(ec == 7),
                )
            nc.vector.tensor_scalar_add(
                kt_all[:, dc, s0 : s0 + SB], pk, bkc_t[:, dc : dc + 1]
            )

        def emit_proj_v(sb, tsub):
            xt_sb = xt_tiles[sb]
            tcg = 4 * sb + tsub
            pv = ppool.tile([128, NHL, DH], f32, tag="pp", name=f"pv_{sb}_{tsub}")
            for ec in range(8):
                nc.tensor.matmul(
                    pv, xt_sb[:, ec, ts(tsub, 128)], wv_t[:, ec, :],
                    start=(ec == 0), stop=(ec == 7),
                )
            nc.vector.tensor_copy(v_aug[:, tcg, :, 0:DH], pv[:, :, :])

        def emit_outproj(sb, sc, oh):
            s0 = sb * SB
            ao = ao_tiles[sb]
            po = ppool.tile([128, 512], f32, tag="pp", name=f"pop_{sb}_{sc}_{oh}")
            for p in range(4):
                nc.tensor.matmul(
                    po,
                    ao[p][:, ts(sc, 128)],
                    wo_t[:, p, ts(oh, 512)],
                    start=(p == 0), stop=(p == 3),
                )
            po_sb = work.tile([128, 512], bf16, tag="posb", bufs=2)
            nc.vector.tensor_copy(po_sb, po)
            nc.sync.dma_start(
                out_d[s0 + 128 * sc : s0 + 128 * (sc + 1), ts(oh, 512)], po_sb
            )

        def proj_items(sb):
            items = []
            for dc in range(4):
                items.append(lambda sb=sb, dc=dc: emit_proj_q(sb, dc))
            for dc in range(4):
                items.append(lambda sb=sb, dc=dc: emit_proj_k(sb, dc))
            for tsub in range(4):
                items.append(lambda sb=sb, tsub=tsub: emit_proj_v(sb, tsub))
            return items

        def outproj_items(sb):
            return [
                lambda sb=sb, sc=sc, oh=oh: emit_outproj(sb, sc, oh)
                for sc in range(4)
                for oh in range(2)
            ]

        # proj(0) runs standalone up front; xt(1) starts loading right after
        for it in proj_items(0):
            it()
        emit_xt_load(1)

        # fill-schedule per attention phase: projections go to the early
        # (exp-light, PE-idle) phases, all out-projections to the exp-bound
        # last phase.
        p3 = proj_items(3)
        fills = {
            0: proj_items(1),
            1: proj_items(2) + p3[:2],
            2: p3[2:],
            3: [],
        }

        for sb in range(NSB):
            s0 = sb * SB
            nkc = 4 * sb + 4
            qt_sb = qt_tiles[sb]

            if sb + 2 < NSB:
                emit_xt_load(sb + 2)
            fill = fills[sb]
            if sb == 3:
                fill = fill + outproj_items(0) + outproj_items(1) + outproj_items(2)
            total_chunks = 4 * nkc
            # even spread: emit fill item i after chunk floor((i+1)*T/(n+1))
            emit_at = [
                (i + 1) * total_chunks // (len(fill) + 1) for i in range(len(fill))
            ]
            fill_i = 0
            chunk_i = 0

            ao_tiles[sb] = []
            for p in range(4):
                out2 = [
                    pout2.tile([DH + 1, SB], f32, tag="po", name=f"out2_{hh}")
                    for hh in range(2)
                ]
                prev = None  # (exp tile, col offset, key chunk)
                for kc in range(nkc):
                    j = kc - 4 * sb  # >= 0 on diagonal chunks
                    c0 = 128 * j if j > 0 else 0
                    ps_t = pscore.tile([128, 2, SB], f32, tag="ps")
                    for hh in range(2):
                        r0 = 64 * hh
                        nc.tensor.matmul(
                            ps_t[:, hh, c0:SB],
                            kt_all[r0 : r0 + 64, p, ts(kc, 128)],
                            qt_sb[r0 : r0 + 64, p, c0:SB],
                            start=True, stop=True,
                        )
                    if prev is not None:
                        pex, pc0, pkc = prev
                        for hh in range(2):
                            nc.tensor.matmul(
                                out2[hh][:, pc0:SB],
                                v_aug[:, pkc, 2 * p + hh, :],
                                pex[:, hh, pc0:SB],
                                start=(pkc == 0), stop=False,
                            )
                    ex = work.tile([128, 2, SB], bf16, tag="expt", bufs=4)
                    nc.scalar.activation(
                        ex[:, :, c0:SB], ps_t[:, :, c0:SB], AF.Exp, scale=SCALE
                    )
                    if j >= 0:
                        # causal mask: zero ex[k, hh, q] where q < k within the
                        # 128x128 diagonal block (iota = col - chan, keep >= 0)
                        nc.gpsimd.affine_select(
                            out=ex[:, :, c0 : c0 + 128],
                            in_=ex[:, :, c0 : c0 + 128],
                            compare_op=mybir.AluOpType.is_ge,
                            fill=0.0,
                            base=0,
                            pattern=[[0, 2], [1, 128]],
                            channel_multiplier=-1,
                        )
                    prev = (ex, c0, kc)
                    chunk_i += 1
                    while fill_i < len(fill) and chunk_i >= emit_at[fill_i]:
                        fill[fill_i]()
                        fill_i += 1
                # final attn@V for the last key chunk
                pex, pc0, pkc = prev
                for hh in range(2):
                    nc.tensor.matmul(
                        out2[hh][:, pc0:SB],
                        v_aug[:, pkc, 2 * p + hh, :],
                        pex[:, hh, pc0:SB],
                        start=(pkc == 0), stop=True,
                    )

                # ---- normalization: broadcast Z, reciprocal, multiply ----
                ao_p = work.tile([128, SB], bf16, tag=f"ao{p}", bufs=4)
                for hh in range(2):
                    z_bf = work.tile([65, SB], bf16, tag="zbf", bufs=2)
                    nc.vector.tensor_copy(z_bf[64:65, :], out2[hh][DH : DH + 1, :])
                    bc_ps = ppool.tile([64, SB], f32, tag="pp", name=f"bcps_{hh}")
                    nc.tensor.matmul(
                        bc_ps[0:64, :], ones_t[64:65, 0:64], z_bf[64:65, :],
                        start=True, stop=True,
                    )
                    rbc = work.tile([64, SB], f32, tag="rbc", bufs=2)
                    nc.vector.reciprocal_approx_fast(rbc, bc_ps[0:64, :])
                    if hh == 0:
                        nc.vector.tensor_mul(ao_p[0:64, :], out2[hh][0:DH, :], rbc)
                    else:
                        aotmp = work.tile([64, SB], bf16, tag="aotmp", bufs=2)
                        nc.vector.tensor_mul(aotmp, out2[hh][0:DH, :], rbc)
                        nc.gpsimd.dma_start(ao_p[64:128, :], aotmp)
                ao_tiles[sb].append(ao_p)

            while fill_i < len(fill):
                fill[fill_i]()
                fill_i += 1

        for it in outproj_items(NSB - 1):
            it()

    nc.compile()
    return nc


def _prepare_core_inputs(x, Wq, bq, Wk, bk, Wv):
    """Build per-core input maps. Core c: b = c // 2, g = c % 2."""
    import ml_dtypes

    BF = ml_dtypes.bfloat16
    maps = []
    xt = [np.ascontiguousarray(x[b].T).astype(BF) for b in range(B)]
    wq_s, wk_s, wv_s, bq_s, bk_s = [], [], [], [], []
    for g in range(HG):
        sl = slice(g * DG, (g + 1) * DG)
        wq_s.append(np.ascontiguousarray(Wq[sl, :].T).astype(BF))
        wk_s.append(np.ascontiguousarray(Wk[sl, :].T).astype(BF))
        wv_s.append(np.ascontiguousarray(Wv[sl, :].T).astype(BF))
        # per-dim bias columns: [128, 4] = bias[dc*128 + r] at [r, dc]
        bq_s.append(np.ascontiguousarray(bq[sl].reshape(4, 128).T).astype(np.float32))
        bk_s.append(np.ascontiguousarray(bk[sl].reshape(4, 128).T).astype(np.float32))
    for c in range(B * HG):
        b, g = c // HG, c % HG
        maps.append({
            "xt": xt[b],
            "wqt": wq_s[g], "wkt": wk_s[g], "wvt": wv_s[g],
            "wot": None,  # filled by caller (needs Wo)
            "bqc": bq_s[g], "bkc": bk_s[g],
        })
    return maps


def kernel(x, Wq, bq, Wk, bk, Wv, bv, Wo, bo):
    from concourse.bass_utils import run_bass_kernel_spmd

    x = np.asarray(x, dtype=np.float32)
    Wq, bq = np.asarray(Wq, np.float32), np.asarray(bq, np.float32)
    Wk, bk = np.asarray(Wk, np.float32), np.asarray(bk, np.float32)
    Wv, bv = np.asarray(Wv, np.float32), np.asarray(bv, np.float32)
    Wo, bo = np.asarray(Wo, np.float32), np.asarray(bo, np.float32)

    if "nc" not in _CACHE:
        _CACHE["nc"] = _build_nc()
    nc = _CACHE["nc"]

    import ml_dtypes

    maps = _prepare_core_inputs(x, Wq, bq, Wk, bk, Wv)
    wot = [
        np.ascontiguousarray(Wo.T[g * DG : (g + 1) * DG, :]).astype(ml_dtypes.bfloat16)
        for g in range(HG)
    ]
    for c in range(B * HG):
        maps[c]["wot"] = wot[c % HG]

    res = run_bass_kernel_spmd(nc, maps, list(range(B * HG)))
    _CACHE["last_results"] = res

    # V-bias folds out of attention exactly (softmax rows sum to 1)
    bo_eff = bo + bv @ Wo.T

    out = np.empty((B, S, D), dtype=np.float32)
    for b in range(B):
        out[b] = (
            res.results[2 * b]["out"].astype(np.float32)
            + res.results[2 * b + 1]["out"].astype(np.float32)
            + bo_eff
        )
    return out
